# revision 36
# baseline (speedup 1.0000x reference)
"""Trainium2 Bass kernel for nn_BasicTransformerBlock (self-attn + cross-attn
+ GEGLU FF, dim=1024, heads=16, seq=4096, ctx=77).

Strategy (8 NeuronCores), v3 (846us, was 930us baseline):
 - Sequence-parallel: each core owns 512 tokens end-to-end, activations kept
   transposed [channel, token] so projections contract over the partition axis.
 - LayerNorm: stats via ones-column matmuls; row stats broadcast to 128
   partitions with K=1 PE matmuls (no DRAM bounce); x-hat materialized with
   two vector ops per tile. LN weight/bias folded into weights/biases host-side.
 - K and V computed first, shipped through ONE combined AllGather issued early;
   Q / cross-attn K2,V2 fill the collective bubble.
 - Softmax: no-max-subtraction exp on ScalarE out of PSUM; denominator rides
   the AV matmul as a ones column; per-pair 1/z broadcast via a K=2 selector
   matmul; division fused into the PSUM->SBUF evacuation.
 - Weights loaded as whole-matrix single DMAs (1-4MB), prefetched a phase
   ahead. FF1/FF2 prefetched during attention phases.
"""
import numpy as np
import ml_dtypes
from contextlib import ExitStack

import concourse.bass as bass
import concourse.tile as tile
import concourse.mybir as mybir
from concourse.bass_utils import run_bass_kernel_spmd


# --- inlined BIR sync-wait legalizer (toolchain accepts max 1 wait/inst) ---
import json as _json


def _legalize_bir_json(raw, max_waits=1):
    d = _json.loads(raw)
    ctr = 0
    for f in d.get("functions", []):
        for bb in f.get("blocks", []):
            out = []
            for ins in bb.get("instructions", []):
                si = ins.get("sync_info")
                if si:
                    waits = si.get("on_wait") or []
                    if len(waits) > max_waits:
                        extra, keep = waits[:-max_waits], waits[-max_waits:]
                        for w in extra:
                            ctr += 1
                            out.append({
                                "debug": ins.get("debug", 0),
                                "engine": ins["engine"],
                                "ins": [],
                                "outs": [],
                                "name": f"waitfix-{ctr}",
                                "opcode": "EventSemaphore",
                                "sync_info": {"on_update": [], "on_wait": [w]},
                            })
                        si["on_wait"] = keep
                    ups = si.get("on_update") or []
                    if len(ups) > 1:
                        raise AssertionError(
                            f"instruction {ins.get('name')} has {len(ups)} updates")
                out.append(ins)
            bb["instructions"] = out
    return _json.dumps(d).encode()


def _install_legalizer(max_waits=1):
    import concourse.bass as _bassmod

    if getattr(_bassmod.Bass, "_legalize_installed", False):
        return
    orig = _bassmod.Bass.to_json_bytes

    def patched(self):
        return _legalize_bir_json(orig(self), max_waits=max_waits)

    _bassmod.Bass.to_json_bytes = patched
    _bassmod.Bass._legalize_installed = True


_install_legalizer()

F32 = mybir.dt.float32
F32R = mybir.dt.float32r
BF16 = mybir.dt.bfloat16
FP8 = mybir.dt.float8e4
DR = mybir.MatmulPerfMode.DoubleRow
AF = mybir.ActivationFunctionType
OP = mybir.AluOpType
WS = 64.0   # fp8 weight scale for the FF matmuls

DIM = 1024
HEADS = 16
D = 64
CTX = 768
FF = 4096
T = 4096
NCORES = 8
TO = T // NCORES          # 512 own tokens per core
KT = T // 128             # 32 k-tiles over full sequence
PAIRS = HEADS // 2        # 8 head pairs
CKT = DIM // 128          # 8 contraction tiles over DIM
CKT_CTX = CTX // 128      # 6 contraction tiles over CTX
TCX = 77
TCXP = 80  # ctx tokens padded
SCALE = D ** -0.5
EPS = 1e-5

# AllGather payload layout (bf16 elements, per rank): K^T block then V block
K_ELEMS = DIM * TO                  # K^T own block [1024, 512]
V_ROW = HEADS * (D + 1)             # 1040: per-token augmented V row
V_ELEMS = TO * V_ROW                # V augmented block [512, 1040]
AG_ELEMS = K_ELEMS + V_ELEMS


def _ap(tensor_ap, offset, steps):
    """Raw AP view on a (flat) dram tensor: steps = [[step, count], ...]."""
    return bass.AP(tensor=tensor_ap.tensor, offset=tensor_ap.offset + offset,
                   ap=list(steps))


def build_nc(fake_ag=False):
    nc = bass.Bass(trn_type="TRN2")

    # ---- dram tensors ----------------------------------------------------
    xT = nc.dram_tensor("xT", [DIM, TO], BF16, kind="ExternalInput")
    ctxT = nc.dram_tensor("ctxT", [CTX, TCXP], BF16, kind="ExternalInput")

    def w_in(name, shape, dt=BF16):
        return nc.dram_tensor(name, list(shape), dt, kind="ExternalInput")

    # big weight layouts: [128, m, kt, n] (lhsT m-tiles) / [128, nb, kt, n] (rhs)
    wq1t = w_in("wq1t", (128, 8, CKT, 128))
    wk1t = w_in("wk1t", (128, 8, CKT, 128))
    wv1t = w_in("wv1t", (128, 2, CKT, 512))
    o1t = w_in("o1t", (128, 8, CKT, 128))
    wq2t = w_in("wq2t", (128, 8, CKT, 128))
    k2t = w_in("k2t", (128, 8, CKT_CTX, 128))
    v2t = w_in("v2t", (128, 2, CKT_CTX, 512))
    o2t = w_in("o2t", (128, 8, CKT, 128))
    # FF weights too big for SBUF residency: streamed per m-tile
    ff1g = w_in("ff1g", (128, 32, CKT, 128))
    ff1a = w_in("ff1a", (128, 32, CKT, 128))
    ff2t = w_in("ff2t", (128, 8, FF // 128, 128))

    qb1c = w_in("qb1c", (128, 8), F32)
    kb1c = w_in("kb1c", (128, 8), F32)
    vrow = w_in("vrow", (1, DIM), BF16)       # v bias as a row
    o1bc = w_in("o1bc", (128, 8), F32)
    qb2c = w_in("qb2c", (128, 8), F32)
    o2bc = w_in("o2bc", (128, 8), F32)
    fb1c = w_in("fb1c", (128, 64), F32)
    padmask = w_in("padmask", (128, 16), F32)
    ff2bc = w_in("ff2bc", (128, 8), F32)

    outT = nc.dram_tensor("outT", [DIM, TO], F32, kind="ExternalOutput")

    with tile.TileContext(nc) as tc, ExitStack() as top:
        dram = top.enter_context(tc.tile_pool(name="dram", bufs=1, space="DRAM"))
        p_const = top.enter_context(tc.tile_pool(name="p_const", bufs=1))
        p_xin = top.enter_context(tc.tile_pool(name="p_xin", bufs=1))

        # x first on the HWDGE ring: everything in phase A waits on it
        xt_all = p_xin.tile([128, CKT, TO], BF16, name="xt_all")
        nc.sync.dma_start(
            out=xt_all,
            in_=_ap(xT.ap(), 0, [[TO, 128], [128 * TO, CKT], [1, TO]]))

        # ---- constants (gpsimd ring: keep the HWDGE ring clear) ----------
        # ones column scaled by 1/DIM -> stats matmuls produce mu, m2 directly
        oneN = p_const.tile([128, 1], BF16, name="oneN")
        nc.vector.memset(oneN[:], 1.0 / DIM)
        ones1r = p_const.tile([1, 128], BF16, name="ones1r")  # K=1 bcast lhsT
        nc.vector.memset(ones1r[:], 1.0)
        ones16 = p_const.tile([128, 16], F32, name="ones16")
        nc.vector.memset(ones16[:], 1.0)
        padones = p_const.tile([128, 16], F32, name="padones")
        nc.gpsimd.dma_start(out=padones[:], in_=padmask.ap())
        eps_row = p_const.tile([1, 1], F32, name="eps_row")
        nc.vector.memset(eps_row[:], EPS)
        # selectors for pair z broadcast (K=1 matmuls): selA -> parts 0-63,
        # selB -> parts 64-127
        selA = p_const.tile([1, 128], BF16, name="selA")
        nc.vector.memset(selA[:], 0.0)
        nc.vector.memset(selA[0:1, 0:64], 1.0)
        selB = p_const.tile([1, 128], BF16, name="selB")
        nc.vector.memset(selB[:], 0.0)
        nc.vector.memset(selB[0:1, 64:128], 1.0)
        vbrow = p_const.tile([1, DIM], BF16, name="vbrow")
        nc.gpsimd.dma_start(out=vbrow[:], in_=vrow.ap())

        def bias_tile(name, dram_t, cols):
            t = p_const.tile([128, cols], F32, name=name)
            nc.gpsimd.dma_start(out=t[:], in_=dram_t.ap())
            return t

        qb1 = bias_tile("qb1", qb1c, 8)
        kb1 = bias_tile("kb1", kb1c, 8)
        o1b = bias_tile("o1b", o1bc, 8)
        qb2 = bias_tile("qb2", qb2c, 8)
        o2b = bias_tile("o2b", o2bc, 8)
        fb1 = bias_tile("fb1", fb1c, 64)
        ff2b = bias_tile("ff2b", ff2bc, 8)

        ctx_sb = []
        for i in range(CKT_CTX):
            t = p_const.tile([128, TCXP], BF16, name=f"ctxsb{i}")
            nc.gpsimd.dma_start(out=t[:], in_=ctxT.ap()[i * 128:(i + 1) * 128, :])
            ctx_sb.append(t)

        # ---- layernorm: produce xhat tiles [128, TO] bf16 ----------------
        def layernorm(xtiles, out_pool, tag, out_aps=None):
            """xtiles: 8 sbuf tiles [128, TO] BF16. Returns 8 xhat tiles
            (BF16 pool tiles, or writes into out_aps if given)."""
            with ExitStack() as ln:
                work = ln.enter_context(tc.tile_pool(name=f"lnw_{tag}", bufs=2))
                rows = ln.enter_context(tc.tile_pool(name=f"lnr_{tag}", bufs=1))
                ps = ln.enter_context(tc.tile_pool(name=f"lnp_{tag}", bufs=1,
                                                   space="PSUM"))
                ps_s = ps.tile([1, TO], F32, name=f"pss_{tag}", tag="s")
                ps_q = ps.tile([1, TO], F32, name=f"psq_{tag}", tag="q")
                for i in range(8):
                    sq = work.tile([128, TO], BF16, name=f"sq_{tag}", tag="sq")
                    nc.vector.tensor_tensor(sq[:], xtiles[i][:], xtiles[i][:],
                                            op=OP.mult)
                    nc.tensor.matmul(ps_s[:], oneN[:], xtiles[i][:],
                                     start=(i == 0), stop=(i == 7))
                    nc.tensor.matmul(ps_q[:], oneN[:], sq[:],
                                     start=(i == 0), stop=(i == 7))
                mu = rows.tile([1, TO], F32, name=f"mu_{tag}")
                nc.vector.tensor_copy(mu[:], ps_s[:])
                var = rows.tile([1, TO], F32, name=f"var_{tag}")
                nc.vector.tensor_tensor(var[:], mu[:], mu[:], op=OP.mult)
                nc.vector.tensor_tensor(var[:], ps_q[:], var[:], op=OP.subtract)
                lv = rows.tile([1, TO], F32, name=f"lv_{tag}")
                nc.scalar.activation(lv[:], var[:], AF.Ln, bias=eps_row[:])
                ra = rows.tile([1, TO], F32, name=f"ra_{tag}")
                nc.scalar.activation(ra[:], lv[:], AF.Exp, scale=-0.5)
                rabf = rows.tile([1, TO], BF16, name=f"rabf_{tag}")
                nc.vector.tensor_copy(rabf[:], ra[:])
                nmubf = rows.tile([1, TO], BF16, name=f"nmubf_{tag}")
                nc.vector.tensor_scalar(nmubf[:], mu[:], -1.0, None, op0=OP.mult)
                # broadcast to 128 partitions via K=1 matmuls
                ps_bc = ps.tile([128, 2, TO], F32, name=f"psbc_{tag}", tag="bc")
                nc.tensor.matmul(ps_bc[:, 0, :], ones1r[:], rabf[:],
                                 start=True, stop=True)
                nc.tensor.matmul(ps_bc[:, 1, :], ones1r[:], nmubf[:],
                                 start=True, stop=True)
                RA = work.tile([128, TO], BF16, name=f"RA_{tag}", tag="RA")
                nc.vector.tensor_copy(RA[:], ps_bc[:, 0, :])
                NMU = work.tile([128, TO], BF16, name=f"NMU_{tag}", tag="NMU")
                nc.vector.tensor_copy(NMU[:], ps_bc[:, 1, :])
                out = []
                for i in range(8):
                    tmp = work.tile([128, TO], BF16, name=f"tmp_{tag}", tag="tmp")
                    nc.vector.tensor_tensor(tmp[:], xtiles[i][:], NMU[:],
                                            op=OP.add)
                    if out_aps is not None:
                        h = out_aps[i]
                    else:
                        h = out_pool.tile([128, TO], BF16, name=f"h_{tag}{i}")
                    nc.vector.tensor_tensor(h[:], tmp[:], RA[:], op=OP.mult)
                    out.append(h)
                return out

        def proj_T(wtile, rhs_tiles, bias, out_pool, tag, nkt=CKT,
                   out_dtype=BF16, residual=None, res_bias=None,
                   cast_pool=None, wdram=None):
            """out^T[m] = sum_kt W[:, m, kt, :].T @ rhs[kt]  (+bias col m).
            If residual given: out = (psum + res_bias_m) + residual[m].
            cast_pool: also emit a BF16 copy of each output tile.
            wdram: stream weights per m-tile from this dram tensor instead."""
            outs = []
            casts = []
            with ExitStack() as st:
                ps = st.enter_context(tc.tile_pool(name=f"ps_{tag}", bufs=2,
                                                   space="PSUM"))
                if wdram is not None:
                    wp = st.enter_context(tc.tile_pool(name=f"wp_{tag}",
                                                       bufs=3))
                for m in range(8):
                    if wdram is not None:
                        wm = wp.tile([128, nkt, 128], BF16, name=f"wm_{tag}",
                                     tag="w")
                        nc.gpsimd.dma_start(out=wm[:], in_=wdram.ap()[:, m, :, :])
                        wv_ = wm
                    psy = ps.tile([128, TO], F32, name=f"psy_{tag}", tag="y")
                    for kt in range(nkt):
                        nc.tensor.matmul(psy[:],
                                         wv_[:, kt, :] if wdram is not None
                                         else wtile[:, m, kt, :],
                                         rhs_tiles[kt][:],
                                         start=(kt == 0), stop=(kt == nkt - 1))
                    o = out_pool.tile([128, TO], out_dtype, name=f"o_{tag}{m}")
                    if residual is not None:
                        nc.vector.scalar_tensor_tensor(
                            o[:], psy[:], res_bias[:, m:m + 1],
                            residual[m][:], op0=OP.add, op1=OP.add)
                    elif bias is not None:
                        nc.vector.tensor_scalar(o[:], psy[:], bias[:, m:m + 1],
                                                None, op0=OP.add)
                    else:
                        nc.vector.tensor_copy(o[:], psy[:])
                    outs.append(o)
                    if cast_pool is not None:
                        c = cast_pool.tile([128, TO], BF16,
                                           name=f"c_{tag}{m}")
                        nc.vector.tensor_copy(c[:], o[:])
                        casts.append(c)
            if cast_pool is not None:
                return outs, casts
            return outs

        # ---- AG buffers: K and V gathered separately, K first ------------
        agkv_in = dram.tile([AG_ELEMS], BF16, name="agkv_in")
        ag_space = "Local" if fake_ag else "Shared"
        agk_out = dram.tile([NCORES * K_ELEMS], BF16, name="agk_out",
                            addr_space=ag_space)
        agv_out = dram.tile([NCORES * V_ELEMS], BF16, name="agv_out",
                            addr_space=ag_space)

        def ag_part(off, n, out_t):
            src_ap = _ap(agkv_in[:], off, [[TO, n // TO], [1, TO]])
            if fake_ag:
                for r in range(NCORES):
                    nc.sync.dma_start(
                        out=_ap(out_t[:], r * n, [[TO, n // TO], [1, TO]]),
                        in_=src_ap)
            else:
                nc.gpsimd.collective_compute(
                    "AllGather", OP.bypass,
                    replica_groups=[list(range(NCORES))],
                    ins=[src_ap], outs=[out_t[:]])

        # ================= phase A ========================================
        p_x3 = top.enter_context(tc.tile_pool(name="p_x3", bufs=1))
        p_x3b = top.enter_context(tc.tile_pool(name="p_x3b", bufs=1))
        sAC = top.enter_context(ExitStack())   # pools alive through phase C
        p_x2 = sAC.enter_context(tc.tile_pool(name="p_x2", bufs=1))
        p_kv2 = sAC.enter_context(tc.tile_pool(name="p_kv2", bufs=1))
        sAB = sAC.enter_context(ExitStack())   # pools alive through o1
        p_xT = sAB.enter_context(tc.tile_pool(name="p_xT", bufs=1))
        p_QT = sAB.enter_context(tc.tile_pool(name="p_QT", bufs=1))
        p_OT = sAB.enter_context(tc.tile_pool(name="p_OT", bufs=1))

        xt = [xt_all[:, i, :] for i in range(CKT)]

        with ExitStack() as phA:
            p_wA = phA.enter_context(tc.tile_pool(name="p_wA", bufs=1))
            # critical-path weights first; the rest are issued after the AG
            wk_sb = p_wA.tile([128, 8, CKT, 128], BF16, name="wk_sb")
            nc.sync.dma_start(out=wk_sb, in_=wk1t.ap())
            wv_sb = p_wA.tile([128, 2, CKT, 512], BF16, name="wv_sb")
            nc.sync.dma_start(out=wv_sb, in_=wv1t.ap())
            wq_sb = p_wA.tile([128, 8, CKT, 128], BF16, name="wq_sb")
            nc.sync.dma_start(out=wq_sb, in_=wq1t.ap())
            p_h1 = phA.enter_context(tc.tile_pool(name="p_h1", bufs=1))
            h1 = layernorm(xt, p_h1, "ln1")

            # K^T -> agkv_in[0 : K_ELEMS) viewed [DIM, TO]
            with ExitStack() as stk:
                ps = stk.enter_context(tc.tile_pool(name="ps_k1", bufs=2,
                                                    space="PSUM"))
                kst = stk.enter_context(tc.tile_pool(name="p_kst", bufs=2))
                for m in range(8):
                    psy = ps.tile([128, TO], F32, name="psy_k1", tag="y")
                    for kt in range(CKT):
                        nc.tensor.matmul(psy[:], wk_sb[:, m, kt, :], h1[kt][:],
                                         start=(kt == 0), stop=(kt == CKT - 1))
                    ko = kst.tile([128, TO], BF16, name="ko_k1", tag="ko")
                    nc.vector.tensor_scalar(ko[:], psy[:], kb1[:, m:m + 1],
                                            None, op0=OP.add)
                    nc.sync.dma_start(
                        out=_ap(agkv_in[:], m * 128 * TO, [[TO, 128], [1, TO]]),
                        in_=ko[:])
                    if m == 7:
                        ag_part(0, K_ELEMS, agk_out)

            # V augmented -> agkv_in[K_ELEMS:) viewed [TO, 1040]
            with ExitStack() as stv:
                ps = stv.enter_context(tc.tile_pool(name="ps_v1", bufs=2,
                                                    space="PSUM"))
                vst = stv.enter_context(tc.tile_pool(name="p_vst", bufs=2))
                for t4 in range(4):
                    vag = vst.tile([128, V_ROW], BF16, name="vag", tag="vag")
                    vag3 = vag.rearrange("p (h e) -> p h e", e=D + 1)
                    for nb in range(2):
                        psv = ps.tile([128, 512], F32, name="psv", tag="v")
                        for kt in range(CKT):
                            nc.tensor.matmul(
                                psv[:], h1[kt][:, t4 * 128:(t4 + 1) * 128],
                                wv_sb[:, nb, kt, :],
                                start=(kt == 0), stop=False)
                        # + v bias broadcast along tokens (K=1 ones matmul)
                        nc.tensor.matmul(
                            psv[:], ones1r[:],
                            vbrow[:, nb * 512:(nb + 1) * 512],
                            start=False, stop=True)
                        nc.vector.tensor_copy(
                            vag3[:, nb * 8:(nb + 1) * 8, 0:D],
                            psv[:].rearrange("p (h e) -> p h e", e=D))
                    nc.scalar.copy(vag3[:, :, D:D + 1], ones16.unsqueeze(2))
                    nc.sync.dma_start(
                        out=_ap(agkv_in[:], K_ELEMS + t4 * 128 * V_ROW,
                                [[V_ROW, 128], [1, V_ROW]]),
                        in_=vag[:])
                    if t4 == 3:
                        ag_part(K_ELEMS, V_ELEMS, agv_out)

            # deferred weight loads (after AG trigger, off the critical path)
            k2_sb = p_wA.tile([128, 8, CKT_CTX, 128], BF16, name="k2_sb")
            nc.sync.dma_start(out=k2_sb, in_=k2t.ap())
            v2_sb = p_wA.tile([128, 2, CKT_CTX, 512], BF16, name="v2_sb")
            nc.sync.dma_start(out=v2_sb, in_=v2t.ap())

            # Q projection + cross-attn K2/V2 fill the collective bubble
            QT = proj_T(wq_sb, h1, qb1, p_QT, "q1")

            K2T = []
            with ExitStack() as stk2:
                ps = stk2.enter_context(tc.tile_pool(name="ps_k2", bufs=2,
                                                     space="PSUM"))
                for m in range(8):
                    psy = ps.tile([128, TCXP], F32, name="psy_k2", tag="y")
                    for kt in range(CKT_CTX):
                        nc.tensor.matmul(psy[:], k2_sb[:, m, kt, :],
                                         ctx_sb[kt][:],
                                         start=(kt == 0),
                                         stop=(kt == CKT_CTX - 1))
                    k2 = p_kv2.tile([128, TCXP], BF16, name=f"k2_{m}")
                    nc.vector.tensor_copy(k2[:], psy[:])
                    K2T.append(k2)

            v2ag = p_kv2.tile([TCXP, V_ROW], BF16, name="v2ag")
            v2ag3 = v2ag.rearrange("p (h e) -> p h e", e=D + 1)
            with ExitStack() as stv2:
                ps = stv2.enter_context(tc.tile_pool(name="ps_v2", bufs=2,
                                                     space="PSUM"))
                for nb in range(2):
                    psv = ps.tile([TCXP, 512], F32, name="psv2", tag="v")
                    for kt in range(CKT_CTX):
                        nc.tensor.matmul(psv[:], ctx_sb[kt][:],
                                         v2_sb[:, nb, kt, :],
                                         start=(kt == 0),
                                         stop=(kt == CKT_CTX - 1))
                    nc.vector.tensor_copy(
                        v2ag3[:, nb * 8:(nb + 1) * 8, 0:D],
                        psv[:].rearrange("p (h e) -> p h e", e=D))
                nc.scalar.copy(v2ag3[:, :, D:D + 1],
                               padones[0:TCXP, :].unsqueeze(2))

        # ================= phase B: self-attention ========================
        with ExitStack() as phB:
            p_at = phB.enter_context(tc.tile_pool(name="p_at", bufs=2))
            p_pt = phB.enter_context(tc.tile_pool(name="p_pt", bufs=3))
            p_vp = phB.enter_context(tc.tile_pool(name="p_vp", bufs=3))
            p_rb = phB.enter_context(tc.tile_pool(name="p_rb", bufs=2))
            ps_S = phB.enter_context(tc.tile_pool(name="ps_S", bufs=2,
                                                  space="PSUM"))
            ps_AV = phB.enter_context(tc.tile_pool(name="ps_AV", bufs=1,
                                                   space="PSUM"))
            ps_BC = phB.enter_context(tc.tile_pool(name="ps_BC", bufs=1,
                                                   space="PSUM"))

            for p in range(PAIRS):
                kpair = p_at.tile([128, T], BF16, name="kpair", tag="kp")
                nc.sync.dma_start(
                    out=kpair.rearrange("p (r t) -> p r t", r=NCORES),
                    in_=_ap(agk_out[:], p * 128 * TO,
                            [[TO, 128], [K_ELEMS, NCORES], [1, TO]]))
                psA = ps_AV.tile([128, TO], F32, name="psA", tag="A")
                psB = ps_AV.tile([128, TO], F32, name="psB", tag="B")
                for kt in range(KT):
                    r, lt = kt // 4, kt % 4
                    if lt == 0:
                        vp4 = p_vp.tile([128, 4, 2 * (D + 1)], BF16,
                                        name="vp4", tag="vp")
                        nc.sync.dma_start(
                            out=vp4[:],
                            in_=_ap(agv_out[:],
                                    r * V_ELEMS + p * 2 * (D + 1),
                                    [[V_ROW, 128], [128 * V_ROW, 4],
                                     [1, 2 * (D + 1)]]))
                    pss = ps_S.tile([128, 2, TO], F32, name="pss", tag="s")
                    nc.tensor.matmul(pss[:, 0, :],
                                     kpair[0:64, kt * 128:(kt + 1) * 128],
                                     QT[p][0:64, :], start=True, stop=True,
                                     tile_position=(0, 0))
                    nc.tensor.matmul(pss[:, 1, :],
                                     kpair[64:128, kt * 128:(kt + 1) * 128],
                                     QT[p][64:128, :], start=True, stop=True,
                                     tile_position=(64, 0))
                    pt = p_pt.tile([128, 2, TO], BF16, name="pt", tag="pt")
                    nc.scalar.activation(pt[:], pss[:], AF.Exp)
                    nc.tensor.matmul(psA[0:D + 1, :], vp4[:, lt, 0:D + 1],
                                     pt[:, 0, :],
                                     start=(kt == 0), stop=(kt == KT - 1))
                    nc.tensor.matmul(psB[0:D + 1, :],
                                     vp4[:, lt, D + 1:2 * (D + 1)],
                                     pt[:, 1, :],
                                     start=(kt == 0), stop=(kt == KT - 1))
                # evacuate PSUM fast (raw AV + z rows), divide in place later
                ot = p_OT.tile([128, TO], BF16, name=f"ot{p}")
                nc.vector.tensor_copy(ot[0:64, :], psA[0:D, :])
                zabf = p_rb.tile([1, TO], BF16, name="zabf", tag="za")
                nc.vector.tensor_copy(zabf[:], psA[D:D + 1, :])
                nc.vector.tensor_copy(ot[64:128, :], psB[0:D, :])
                zbbf = p_rb.tile([1, TO], BF16, name="zbbf", tag="zb")
                nc.vector.tensor_copy(zbbf[:], psB[D:D + 1, :])
                psbc = ps_BC.tile([128, TO], F32, name="psbc", tag="bc")
                nc.tensor.matmul(psbc[:], selA[:], zabf[:], start=True,
                                 stop=False)
                nc.tensor.matmul(psbc[:], selB[:], zbbf[:], start=False,
                                 stop=True)
                rec = p_rb.tile([128, TO], F32, name="rec", tag="bcs")
                nc.vector.reciprocal(rec[:], psbc[:])
                nc.vector.tensor_tensor(ot[:], ot[:], rec[:], op=OP.mult)
                if p == 0:
                    OT = []
                OT.append(ot)

        # o1 projection + residual -> x2 (bf16)
        x2b = proj_T(None, OT, None, p_x2, "o1", residual=xt,
                     res_bias=o1b, wdram=o1t)
        sAB.close()   # free xt/QT/OT SBUF

        # ================= phase C: cross-attention =======================
        with ExitStack() as phC:
            p_Q2 = phC.enter_context(tc.tile_pool(name="p_Q2", bufs=1))
            p_OT2 = phC.enter_context(tc.tile_pool(name="p_OT2", bufs=1))

            with ExitStack() as stc:
                p_h2 = stc.enter_context(tc.tile_pool(name="p_h2", bufs=1))
                h2 = layernorm(x2b, p_h2, "ln2")
                Q2T = proj_T(None, h2, qb2, p_Q2, "q2", wdram=wq2t)

            with ExitStack() as stx:
                p_rb2 = stx.enter_context(tc.tile_pool(name="p_rb2", bufs=2))
                p_pt2 = stx.enter_context(tc.tile_pool(name="p_pt2", bufs=2))
                ps_S2 = stx.enter_context(tc.tile_pool(name="ps_S2", bufs=1,
                                                       space="PSUM"))
                ps_A2 = stx.enter_context(tc.tile_pool(name="ps_A2", bufs=2,
                                                       space="PSUM"))
                ps_BC2 = stx.enter_context(tc.tile_pool(name="ps_BC2", bufs=1,
                                                        space="PSUM"))
                OT2 = []
                for p in range(PAIRS):
                    pss = ps_S2.tile([TCXP, 2, TO], F32, name="pss2", tag="s")
                    nc.tensor.matmul(pss[:, 0, :], K2T[p][0:64, :],
                                     Q2T[p][0:64, :],
                                     start=True, stop=True, tile_position=(0, 0))
                    nc.tensor.matmul(pss[:, 1, :], K2T[p][64:128, :],
                                     Q2T[p][64:128, :],
                                     start=True, stop=True, tile_position=(64, 0))
                    pt = p_pt2.tile([TCXP, 2, TO], BF16, name="pt2", tag="pt")
                    nc.scalar.activation(pt[:], pss[:], AF.Exp)
                    psA = ps_A2.tile([128, TO], F32, name="psA2", tag="A")
                    psB = ps_A2.tile([128, TO], F32, name="psB2", tag="B")
                    nc.tensor.matmul(psA[0:D + 1, :],
                                     v2ag[:, (2 * p) * (D + 1):(2 * p + 1) * (D + 1)],
                                     pt[:, 0, :], start=True, stop=True)
                    nc.tensor.matmul(psB[0:D + 1, :],
                                     v2ag[:, (2 * p + 1) * (D + 1):(2 * p + 2) * (D + 1)],
                                     pt[:, 1, :], start=True, stop=True)
                    ot = p_OT2.tile([128, TO], BF16, name=f"ot2_{p}")
                    nc.vector.tensor_copy(ot[0:64, :], psA[0:D, :])
                    zabf = p_rb2.tile([1, TO], BF16, name="zabf2", tag="za")
                    nc.vector.tensor_copy(zabf[:], psA[D:D + 1, :])
                    nc.vector.tensor_copy(ot[64:128, :], psB[0:D, :])
                    zbbf = p_rb2.tile([1, TO], BF16, name="zbbf2", tag="zb")
                    nc.vector.tensor_copy(zbbf[:], psB[D:D + 1, :])
                    psbc = ps_BC2.tile([128, TO], F32, name="psbc2", tag="bc")
                    nc.tensor.matmul(psbc[:], selA[:], zabf[:], start=True,
                                     stop=False)
                    nc.tensor.matmul(psbc[:], selB[:], zbbf[:], start=False,
                                     stop=True)
                    rec = p_rb2.tile([128, TO], F32, name="rec2", tag="bcs")
                    nc.vector.reciprocal(rec[:], psbc[:])
                    nc.vector.tensor_tensor(ot[:], ot[:], rec[:], op=OP.mult)
                    OT2.append(ot)

            x3f, x3b = proj_T(None, OT2, None, p_x3, "o2", residual=x2b,
                              res_bias=o2b, out_dtype=F32, cast_pool=p_x3b,
                              wdram=o2t)
        sAC.close()   # free x2/K2/V2/phase-C weight SBUF

        # ================= phase D: GEGLU FF ==============================
        with ExitStack() as phD:
            p_hT = phD.enter_context(tc.tile_pool(name="p_hT", bufs=1))
            hT = []
            with ExitStack() as stf:
                p_h3 = stf.enter_context(tc.tile_pool(name="p_h3", bufs=1))
                h3 = layernorm(x3b, p_h3, "ln3")
                wp = stf.enter_context(tc.tile_pool(name="wp_ff1", bufs=6))
                gp = stf.enter_context(tc.tile_pool(name="p_g", bufs=2))
                ps = stf.enter_context(tc.tile_pool(name="ps_ff1", bufs=3,
                                                    space="PSUM"))
                for i in range(32):
                    wg = wp.tile([128, CKT, 128], BF16, name="wg_ff1", tag="w")
                    nc.gpsimd.dma_start(out=wg[:], in_=ff1g.ap()[:, i, :, :])
                    psg = ps.tile([128, TO], F32, name="psg", tag="p")
                    for kt in range(CKT):
                        nc.tensor.matmul(psg[:], wg[:, kt, :], h3[kt][:],
                                         start=(kt == 0), stop=(kt == CKT - 1))
                    g = gp.tile([128, TO], F32, name="g", tag="g")
                    nc.scalar.activation(g[:], psg[:], AF.Gelu,
                                         bias=fb1[:, 32 + i:33 + i], scale=1.0)
                    wa = wp.tile([128, CKT, 128], BF16, name="wa_ff1", tag="w")
                    nc.gpsimd.dma_start(out=wa[:], in_=ff1a.ap()[:, i, :, :])
                    psa = ps.tile([128, TO], F32, name="psa", tag="p")
                    for kt in range(CKT):
                        nc.tensor.matmul(psa[:], wa[:, kt, :], h3[kt][:],
                                         start=(kt == 0), stop=(kt == CKT - 1))
                    h = p_hT.tile([128, TO], BF16, name=f"hT{i}")
                    nc.vector.scalar_tensor_tensor(h[:], psa[:], fb1[:, i:i + 1],
                                                   g[:], op0=OP.add, op1=OP.mult)
                    hT.append(h)

            with ExitStack() as stf2:
                wp2 = stf2.enter_context(tc.tile_pool(name="wp_ff2", bufs=3))
                outp = stf2.enter_context(tc.tile_pool(name="p_out", bufs=2))
                ps = stf2.enter_context(tc.tile_pool(name="ps_ff2", bufs=2,
                                                     space="PSUM"))
                for m in range(8):
                    wm = wp2.tile([128, FF // 128, 128], BF16, name="wm_ff2",
                                  tag="w")
                    nc.gpsimd.dma_start(out=wm[:], in_=ff2t.ap()[:, m, :, :])
                    psy = ps.tile([128, TO], F32, name="psy_ff2", tag="y")
                    for kt in range(FF // 128):
                        nc.tensor.matmul(psy[:], wm[:, kt, :], hT[kt][:],
                                         start=(kt == 0),
                                         stop=(kt == FF // 128 - 1))
                    o = outp.tile([128, TO], F32, name="of", tag="of")
                    nc.vector.scalar_tensor_tensor(o[:], psy[:], ff2b[:, m:m + 1],
                                                   x3f[m][:],
                                                   op0=OP.add, op1=OP.add)
                    nc.sync.dma_start(out=outT.ap()[m * 128:(m + 1) * 128, :],
                                      in_=o[:])

    return nc


# ---------------------------------------------------------------------------
# host side
# ---------------------------------------------------------------------------
def _tile_lhs(w, nm, nkt):
    """[K, M] -> [128, nm, nkt, 128] with [p][m][kt][n] = w[kt*128+p, m*128+n]."""
    K, M = w.shape
    assert K == nkt * 128 and M == nm * 128
    return np.ascontiguousarray(
        w.reshape(nkt, 128, nm, 128).transpose(1, 2, 0, 3))


def _tile_rhs(w, nkt):
    """[K, N] -> [128, N//512, nkt, 512] with [p][nb][kt][n] = w[kt*128+p, nb*512+n]."""
    K, N = w.shape
    assert K == nkt * 128 and N % 512 == 0
    return np.ascontiguousarray(
        w.reshape(nkt, 128, N // 512, 512).transpose(1, 2, 0, 3))


def _bias_cols(b, ncols):
    return np.ascontiguousarray(np.asarray(b, np.float32).reshape(ncols, 128).T)


def _tile_lhs_fp8(w, nm, nkt):
    """[K, M] -> [128, nm, nkt//2, 2, 128] k-pair interleaved, scaled by WS."""
    K, M = w.shape
    assert K == nkt * 128 and M == nm * 128 and nkt % 2 == 0
    t = (w * WS).reshape(nkt // 2, 2, 128, nm, 128).transpose(2, 3, 0, 1, 4)
    return np.ascontiguousarray(np.clip(t, -240.0, 240.0))


_NC_CACHE = None


def kernel(**inputs):
    global _NC_CACHE
    inp = {k: np.asarray(v, np.float32) for k, v in inputs.items()}

    x = inp["x"][0]                    # [T, DIM]
    ctx = inp["context"][0]            # [77, CTX]
    xT_full = np.ascontiguousarray(x.T)
    ctxT = np.zeros((CTX, TCXP), np.float32)
    ctxT[:, :TCX] = ctx.T

    wq1 = np.ascontiguousarray((inp["n1_w"][:, None] * inp["q1_w"]) * SCALE)
    wk1 = np.ascontiguousarray(inp["n1_w"][:, None] * inp["k1_w"])
    wv1 = np.ascontiguousarray(inp["n1_w"][:, None] * inp["v1_w"])
    qb1 = (inp["n1_b"] @ inp["q1_w"]) * SCALE
    kb1 = inp["n1_b"] @ inp["k1_w"]
    vb1 = inp["n1_b"] @ inp["v1_w"]
    wq2 = np.ascontiguousarray((inp["n2_w"][:, None] * inp["q2_w"]) * SCALE)
    qb2 = (inp["n2_b"] @ inp["q2_w"]) * SCALE
    ff1 = np.ascontiguousarray(inp["n3_w"][:, None] * inp["ff1_w"])
    fb1 = inp["n3_b"] @ inp["ff1_w"] + inp["ff1_b"]

    shared = {
        "ctxT": ctxT,
        "wq1t": _tile_lhs(wq1, 8, CKT),
        "wk1t": _tile_lhs(wk1, 8, CKT),
        "wv1t": _tile_rhs(wv1, CKT),
        "o1t": _tile_lhs(np.ascontiguousarray(inp["o1_w"]), 8, CKT),
        "wq2t": _tile_lhs(wq2, 8, CKT),
        "k2t": _tile_lhs(np.ascontiguousarray(inp["k2_w"]), 8, CKT_CTX),
        "v2t": _tile_rhs(np.ascontiguousarray(inp["v2_w"]), CKT_CTX),
        "o2t": _tile_lhs(np.ascontiguousarray(inp["o2_w"]), 8, CKT),
        "ff1g": _tile_lhs(np.ascontiguousarray(ff1[:, FF:]), 32, CKT),
        "ff1a": _tile_lhs(np.ascontiguousarray(ff1[:, :FF]), 32, CKT),
        "ff2t": _tile_lhs(np.ascontiguousarray(inp["ff2_w"]), 8, FF // 128),
        "vrow": np.ascontiguousarray(vb1.reshape(1, DIM)),
        "qb1c": _bias_cols(qb1, 8),
        "kb1c": _bias_cols(kb1, 8),
        "o1bc": _bias_cols(inp["o1_b"], 8),
        "qb2c": _bias_cols(qb2, 8),
        "o2bc": _bias_cols(inp["o2_b"], 8),
        "fb1c": _bias_cols(fb1, 64),
        "padmask": np.ascontiguousarray(
            (np.arange(128)[:, None] < TCX).astype(np.float32)
            * np.ones((1, 16), np.float32)),
        "ff2bc": _bias_cols(inp["ff2_b"], 8),
    }
    f32_keys = {"qb1c", "kb1c", "o1bc", "qb2c", "o2bc", "fb1c",
                "ff2bc", "padmask"}
    fp8_keys = set()

    def _dt(k):
        if k in f32_keys:
            return np.float32
        if k in fp8_keys:
            return ml_dtypes.float8_e4m3
        return ml_dtypes.bfloat16

    shared = {k: np.ascontiguousarray(v, dtype=_dt(k))
              for k, v in shared.items()}

    in_maps = []
    for c in range(NCORES):
        m = dict(shared)
        m["xT"] = np.ascontiguousarray(
            xT_full[:, c * TO:(c + 1) * TO], dtype=ml_dtypes.bfloat16)
        in_maps.append(m)

    if _NC_CACHE is None:
        _NC_CACHE = build_nc()
    nc = _NC_CACHE

    res = run_bass_kernel_spmd(nc, in_maps, core_ids=list(range(NCORES)))

    outs = [res.results[c]["outT"].T for c in range(NCORES)]   # each [TO, DIM]
    return np.ascontiguousarray(np.concatenate(outs, axis=0))[None].astype(np.float32)


if __name__ == "__main__":
    d = np.load("/tmp/ref_inputs.npz")
    out = kernel(**{k: d[k] for k in d.files})
    ref = np.load("/tmp/ref_out.npy")
    err = np.abs(out - ref).max()
    print("max abs err:", err, " absmax ref:", np.abs(ref).max(),
          " rel:", err / np.abs(ref).max())
